# revision 9
# baseline (speedup 1.0000x reference)
"""Trainium2 Bass kernel for BasicS2Conv (8-core SPMD).

out[b,d,p,r] = sum_{c,k,a} W[d,c,kidx[k,a,r],aidx[k,a,r]] * x[b,c,k,p,a]
with B=4, C=D=128, K=13, A=R=12, P=1024.

Sharding: data-parallel over p across the 8 NeuronCores (128 p's each,
all of b on every core); W is replicated. No collectives needed.

Per-core algorithm:
- The contraction maps to 128x128x512 matmuls: for (k, a, r),
  stationary = W[:, :, kidx, aidx] as [c, d], moving = x[:, :, k, :, a]
  as [c, (b,p)] N=512 columns, accumulated into the PSUM slice for r.
  Both operands in fp32r (rounded fp32, ~13-bit mantissa) which streams
  at 1 column/cycle (plain fp32 is 4x slower).
- x is host-pre-transposed to [C, K, A, B, PSH] so every DMA is fully
  contiguous and every matmul rhs slice is contiguous (a strided rhs
  runs ~2.9x slower).
- r is processed in two passes of 6 (PSUM holds 6 full banks of 512
  fp32); x is streamed from HBM once per pass.
- Gather redundancy: several (k,a) pairs share one weight slice
  (kk,aa) for a given r. For the highest-multiplicity bins the x
  slices are pre-summed on the otherwise-idle Vector engine into SBUF
  accumulators, and a single matmul per bin replaces mult matmuls,
  cutting PE columns by sum(mult-1). NOFF bins per r, bounded by SBUF.

kidx/aidx values are read on the host at trace time and baked into the
instruction stream (programs are cached by index content).
"""

import sys

if "/opt/trn_rl_repo" not in sys.path:
    sys.path.insert(0, "/opt/trn_rl_repo")

from contextlib import ExitStack

import numpy as np

B, C, D, K, A, R, P = 4, 128, 128, 13, 12, 12, 1024
NCORES = 8
PSH = P // NCORES
KK = 5
NOFF = 7  # offloaded bins per r (SBUF-bounded)
MINMULT = 3  # only offload bins with at least this many sources

_cache: dict = {}


def _plan(kidx, aidx):
    """Per r: split the K*A (k,a) pairs into direct matmuls and
    offloaded bins (same (kk,aa), x slices pre-summed on DVE)."""
    plan = []
    for r in range(R):
        bins: dict = {}
        for k in range(K):
            for a in range(A):
                bins.setdefault(
                    (int(kidx[k, a, r]), int(aidx[k, a, r])), []
                ).append((k, a))
        cand = sorted(bins.items(), key=lambda kv: len(kv[1]), reverse=True)
        offload = [
            (kkaa, srcs) for kkaa, srcs in cand[:NOFF] if len(srcs) >= MINMULT
        ]
        off_set = set()
        for _, srcs in offload:
            off_set.update(srcs)
        direct = [
            (k, a, int(kidx[k, a, r]), int(aidx[k, a, r]))
            for k in range(K)
            for a in range(A)
            if (k, a) not in off_set
        ]
        plan.append((direct, offload))
    return plan


def _build(kidx: np.ndarray, aidx: np.ndarray, iters: int = 1):
    import concourse.bass as bass  # noqa: F401
    import concourse.tile as tile
    from concourse import bacc, mybir

    f32 = mybir.dt.float32
    f32r = mybir.dt.float32r

    plan = _plan(kidx, aidx)

    nc = bacc.Bacc(
        "TRN2", target_bir_lowering=False, debug=False, num_devices=NCORES
    )
    x_dram = nc.dram_tensor("x", [C, K, A, B, PSH], f32r, kind="ExternalInput").ap()
    w_dram = nc.dram_tensor("w", [C, KK, A, D], f32r, kind="ExternalInput").ap()
    out_dram = nc.dram_tensor("out", [B, D, PSH, R], f32, kind="ExternalOutput").ap()

    with tile.TileContext(nc) as tc, ExitStack() as ctx:
        wpool = ctx.enter_context(tc.tile_pool(name="wpool", bufs=1))
        xpool = ctx.enter_context(tc.tile_pool(name="xpool", bufs=2))
        opool = ctx.enter_context(tc.tile_pool(name="opool", bufs=1))
        bpool = ctx.enter_context(tc.tile_pool(name="bpool", bufs=1))
        ppool = ctx.enter_context(tc.tile_pool(name="ppool", bufs=1, space="PSUM"))

        w_t = wpool.tile([C, KK, A, D], f32r)
        nc.sync.dma_start(w_t[:], w_dram[:])

        psum = ppool.tile([D, 6, B, PSH], f32)  # 6 banks of 512 fp32
        bins_t = bpool.tile([C, 6, NOFF, B, PSH], f32r)
        out_s = opool.tile([D, B, PSH, R], f32)

        for _ in range(iters):
            for h in range(2):
                rs = [6 * h + j for j in range(6)]
                direct_by_k = [[[] for _ in range(K)] for _ in range(6)]
                for j, r in enumerate(rs):
                    for (k, a, kk, aa) in plan[r][0]:
                        direct_by_k[j][k].append((a, kk, aa))
                first_direct = []
                last_direct = []
                for j in range(6):
                    seq = [
                        (k, i)
                        for k in range(K)
                        for i in range(len(direct_by_k[j][k]))
                    ]
                    first_direct.append(seq[0])
                    last_direct.append(seq[-1])

                for ki in range(K):
                    x_t = xpool.tile([C, A, B, PSH], f32r, tag="xt")
                    nc.sync.dma_start(x_t[:], x_dram[:, ki])
                    for j, r in enumerate(rs):
                        has_bins = len(plan[r][1]) > 0
                        for i, (a, kk, aa) in enumerate(direct_by_k[j][ki]):
                            nc.tensor.matmul(
                                psum[:, j, :, :],
                                w_t[:, kk, aa, :],
                                x_t[:, a, :, :],
                                start=((ki, i) == first_direct[j]),
                                stop=(
                                    not has_bins and (ki, i) == last_direct[j]
                                ),
                            )
                        # DVE: fold this k-tile's sources into bins
                        for bi, (kkaa, srcs) in enumerate(plan[r][1]):
                            for (k, a) in srcs:
                                if k != ki:
                                    continue
                                if (k, a) == srcs[0]:
                                    nc.vector.tensor_copy(
                                        bins_t[:, j, bi, :, :], x_t[:, a, :, :]
                                    )
                                else:
                                    nc.vector.tensor_add(
                                        bins_t[:, j, bi, :, :],
                                        bins_t[:, j, bi, :, :],
                                        x_t[:, a, :, :],
                                    )

                # pass tail: one matmul per offloaded bin
                for j, r in enumerate(rs):
                    nbins = len(plan[r][1])
                    for bi, ((kk, aa), srcs) in enumerate(plan[r][1]):
                        nc.tensor.matmul(
                            psum[:, j, :, :],
                            w_t[:, kk, aa, :],
                            bins_t[:, j, bi, :, :],
                            start=False,
                            stop=(bi == nbins - 1),
                        )

                for j in range(6):
                    nc.vector.tensor_copy(
                        out_s[:, :, :, 6 * h + j], psum[:, j, :, :]
                    )

            for b in range(B):
                nc.sync.dma_start(out_dram[b], out_s[:, b, :, :])

    nc.compile()
    return nc


def kernel(x, W, kidx, aidx):
    from concourse.bass_utils import run_bass_kernel_spmd

    x = np.asarray(x, dtype=np.float32)
    W = np.asarray(W, dtype=np.float32)
    kidx = np.asarray(kidx, dtype=np.int32)
    aidx = np.asarray(aidx, dtype=np.int32)

    key = (kidx.tobytes(), aidx.tobytes())
    nc = _cache.get(key)
    if nc is None:
        nc = _build(kidx, aidx)
        _cache[key] = nc

    # W (d, c, kk, aa) -> (c, kk, aa, d): stationary [c, d] slices contiguous
    w_perm = np.ascontiguousarray(W.transpose(1, 2, 3, 0))
    in_maps = []
    for i in range(NCORES):
        # (B, C, K, PSH, A) -> (C, K, A, B, PSH)
        xs = np.ascontiguousarray(
            x[:, :, :, i * PSH : (i + 1) * PSH, :].transpose(1, 2, 4, 0, 3)
        )
        in_maps.append({"x": xs, "w": w_perm})

    res = run_bass_kernel_spmd(nc, in_maps, list(range(NCORES)))
    out = np.concatenate([res.results[i]["out"] for i in range(NCORES)], axis=2)
    return out
